# revision 1
# baseline (speedup 1.0000x reference)
"""Trainium2 Bass kernel for nn_CausalFullAttention_13735305413109.

Causal attention with a data-dependent cumprod decay gate and no softmax.
Because there is no softmax, the masked quadratic attention is algebraically
a chunked linear attention:
    out_i = q'_i @ State_{blk(i)} + sum_{j<=i, same blk} (q'_i.k'_j) v_j
    State_t = sum_{j < t*BLK} k'_j (x) v_j
with q' = q*SCALE*a_cum, k' = k/max(a_cum,1e-8), computed per (batch, head).

Sharding: head-parallel across 8 cores (head h -> core h, both batches local),
no cross-device communication; each core emits its partial output projection
out_h @ w_out[h*64:(h+1)*64, :] over all 4096 token rows, and the host sums
the 8 partials (+ b_out) as the unshard step.
"""
import numpy as np
from contextlib import ExitStack

import concourse.bass as bass
import concourse.bacc as bacc
import concourse.mybir as mybir
import concourse.tile as tile
from concourse.bass_utils import run_bass_kernel_spmd

F32 = mybir.dt.float32
AF = mybir.ActivationFunctionType
ALU = mybir.AluOpType

B = 2
N = 2048
DIM = 512
HEADS = 8
DH = 64
NTOK = B * N            # 4096 token rows
BLK = 128               # token block
NBLK = N // BLK         # 16 blocks per batch
PANEL = 512             # projection panel (moving free dim)
NPAN = NTOK // PANEL    # 8 panels
NCHUNK = DIM // 128     # 4 contraction chunks
SCALE = DH ** -0.5
LOG_SQRT_DIM = float(np.log(np.sqrt(DIM)))
EPS_INV = 1e-8


def build_nc(with_qkv_bias: bool):
    nc = bacc.Bacc()
    xT_d = nc.dram_tensor("xT", [DIM, NTOK], F32, kind="ExternalInput")
    wqk_d = nc.dram_tensor("wqk", [128, NCHUNK, 128], F32, kind="ExternalInput")
    wvz_d = nc.dram_tensor("wvz", [128, NCHUNK, 128], F32, kind="ExternalInput")
    wout_d = nc.dram_tensor("wout", [DH, DIM], F32, kind="ExternalInput")
    ba_d = nc.dram_tensor("ba", [128, 1], F32, kind="ExternalInput")
    nba_d = nc.dram_tensor("nba", [128, 1], F32, kind="ExternalInput")
    ident_d = nc.dram_tensor("ident", [128, 128], F32, kind="ExternalInput")
    mask_d = nc.dram_tensor("mask", [128, 128], F32, kind="ExternalInput")
    if with_qkv_bias:
        bqk_d = nc.dram_tensor("bqk", [128, 1], F32, kind="ExternalInput")
        bv_d = nc.dram_tensor("bv", [DH, 1], F32, kind="ExternalInput")
    y_d = nc.dram_tensor("ypart", [NTOK, DIM], F32, kind="ExternalOutput")

    with tile.TileContext(nc) as tc, ExitStack() as ctx:
        # ---- persistent sbuf ----
        per = ctx.enter_context(tc.tile_pool(name="persist", bufs=1))
        wqk_sb = per.tile([128, NCHUNK, 128], F32, tag="wqk")
        wvz_sb = per.tile([128, NCHUNK, 128], F32, tag="wvz")
        wout_sb = per.tile([DH, DIM], F32, tag="wout")
        ident_sb = per.tile([128, 128], F32, tag="ident")
        mask_sb = per.tile([128, 128], F32, tag="mask")
        ba_sb = per.tile([128, 1], F32, tag="ba")
        nba_sb = per.tile([128, 1], F32, tag="nba")
        ones_sb = per.tile([128, 128], F32, tag="ones")
        sRep = per.tile([128, NTOK], F32, tag="sRep")
        qk_sb = per.tile([128, NTOK], F32, tag="qk")      # rows 0:64 q'T, 64:128 k'T
        v_sb = per.tile([128, NTOK], F32, tag="v")        # rows 0:64 v'; 64:128 zs then k'
        qdup = per.tile([128, NTOK], F32, tag="qdup")     # rows 64:128 q' copy
        # batch-stacked decay pipeline tiles: rows 0:64 = batch0, 64:128 = batch1
        zstk = per.tile([128, N], F32, tag="zstk")    # z*s; later ainv (scan2 out)
        astk = per.tile([128, N], F32, tag="astk")    # sigmoid; later acum_b1 shift
        estk = per.tile([128, N], F32, tag="estk")    # 1+exp(-z); later ainv_b0 shift
        acstk = per.tile([128, N], F32, tag="acstk")  # cumprod(a)

        nc.sync.dma_start(wqk_sb[:], wqk_d[:])
        nc.sync.dma_start(wvz_sb[:], wvz_d[:])
        nc.sync.dma_start(wout_sb[:], wout_d[:])
        nc.sync.dma_start(ident_sb[:], ident_d[:])
        nc.sync.dma_start(mask_sb[:], mask_d[:])
        nc.sync.dma_start(ba_sb[:], ba_d[:])
        nc.sync.dma_start(nba_sb[:], nba_d[:])
        if with_qkv_bias:
            bqk_sb = per.tile([128, 1], F32, tag="bqk")
            bv_sb = per.tile([128, 1], F32, tag="bv")
            nc.sync.dma_start(bqk_sb[:], bqk_d[:])
            nc.sync.dma_start(bv_sb[0:64, :], bv_d[:])
        nc.gpsimd.memset(ones_sb[:], 1.0)
        lsd_sb = per.tile([128, 1], F32, tag="lsd")
        nc.gpsimd.memset(lsd_sb[:], LOG_SQRT_DIM)

        # ---- phase A: load x, sumsq->s, projections ----
        with (
            tc.tile_pool(name="xt", bufs=2) as xtp,
            tc.tile_pool(name="x2", bufs=4) as x2p,
            tc.tile_pool(name="lns", bufs=2) as lnp,
            tc.tile_pool(name="ss_ps", bufs=2, space="PSUM") as ssp,
            tc.tile_pool(name="vz_ps", bufs=2, space="PSUM") as vzp,
            tc.tile_pool(name="qk_ps", bufs=2, space="PSUM") as qkp,
        ):
            for p in range(NPAN):
                cols = bass.ts(p, PANEL)
                xt = []
                for c in range(NCHUNK):
                    xc = xtp.tile([128, PANEL], F32, tag=f"xt{c}")
                    nc.sync.dma_start(xc[:], xT_d[128 * c:128 * (c + 1), cols])
                    xt.append(xc)
                # sum of squares -> replicated on all partitions via all-ones lhsT
                ss_ps = ssp.tile([128, PANEL], F32)
                for c in range(NCHUNK):
                    x2 = x2p.tile([128, PANEL], F32)
                    if c == 0:
                        nc.scalar.square(x2[:], xt[c][:])
                    elif c == 1:
                        nc.vector.tensor_mul(x2[:], xt[c][:], xt[c][:])
                    else:
                        nc.gpsimd.tensor_mul(x2[:], xt[c][:], xt[c][:])
                    nc.tensor.matmul(ss_ps[:], ones_sb[:], x2[:],
                                     start=(c == 0), stop=(c == NCHUNK - 1))
                # s = exp(-0.5*ln(ss) + ln(sqrt(DIM)))  (= sqrt(DIM)/||x_t||)
                lnt = lnp.tile([128, PANEL], F32)
                nc.scalar.activation(lnt[:], ss_ps[:], AF.Ln)
                nc.scalar.activation(sRep[:, cols], lnt[:], AF.Exp,
                                     bias=lsd_sb[:], scale=-0.5)
                # v|z projection, scaled by s at psum->sbuf
                vz_ps = vzp.tile([128, PANEL], F32)
                for c in range(NCHUNK):
                    nc.tensor.matmul(vz_ps[:], wvz_sb[:, c, :], xt[c][:],
                                     start=(c == 0), stop=(c == NCHUNK - 1))
                nc.vector.tensor_mul(v_sb[:, cols], vz_ps[:], sRep[:, cols])
                if with_qkv_bias:
                    nc.vector.tensor_scalar_add(v_sb[0:64, cols], v_sb[0:64, cols],
                                                bv_sb[0:64, :])
                # q|k projection, scaled by s at psum->sbuf
                qk_ps = qkp.tile([128, PANEL], F32)
                for c in range(NCHUNK):
                    nc.tensor.matmul(qk_ps[:], wqk_sb[:, c, :], xt[c][:],
                                     start=(c == 0), stop=(c == NCHUNK - 1))
                nc.vector.tensor_mul(qk_sb[:, cols], qk_ps[:], sRep[:, cols])
                if with_qkv_bias:
                    nc.vector.tensor_scalar_add(qk_sb[:, cols], qk_sb[:, cols],
                                                bqk_sb[:])

        # ---- phase B: decay gate, both batches stacked on the partition axis ----
        H0, H1, FB = slice(0, 64), slice(64, 128), slice(0, N)
        C0, C1 = slice(0, N), slice(N, 2 * N)
        # zstk rows 0:64 = zs(b0), rows 64:128 = zs(b1)
        nc.sync.dma_start(zstk[H0, FB], v_sb[H1, C0])
        nc.sync.dma_start(zstk[H1, FB], v_sb[H1, C1])
        nc.scalar.activation(astk[:], zstk[:], AF.Sigmoid, bias=ba_sb[:])
        nc.scalar.activation(estk[:], zstk[:], AF.Exp, bias=nba_sb[:], scale=-1.0)
        nc.vector.tensor_scalar_add(estk[:], estk[:], 1.0)
        nc.vector.tensor_tensor_scan(acstk[:], astk[:], astk[:], 1.0,
                                     ALU.mult, ALU.bypass)
        # ainv = min(cumprod(1+exp(-z)), 1e8) == 1/max(cumprod(a), 1e-8)
        nc.vector.tensor_tensor_scan(zstk[:], estk[:], estk[:], 1.0,
                                     ALU.mult, ALU.bypass)
        nc.vector.tensor_scalar_min(zstk[:], zstk[:], 1.0 / EPS_INV)
        # partition shifts so each consumer sees its operand on its own lanes
        nc.sync.dma_start(estk[H1, FB], zstk[H0, FB])    # ainv(b0) -> rows 64:
        nc.sync.dma_start(astk[H0, FB], acstk[H1, FB])   # acum(b1) -> rows 0:
        # q' = q * s * a_cum ; k' = k * s * ainv (k' written into v_sb rows 64:
        # so that one PE transpose per block yields both v'tm and k'tm)
        nc.vector.tensor_mul(qk_sb[H0, C0], qk_sb[H0, C0], acstk[H0, FB])
        nc.vector.tensor_mul(qk_sb[H0, C1], qk_sb[H0, C1], astk[H0, FB])
        nc.vector.tensor_mul(v_sb[H1, C0], qk_sb[H1, C0], estk[H1, FB])
        nc.vector.tensor_mul(v_sb[H1, C1], qk_sb[H1, C1], zstk[H1, FB])
        nc.sync.dma_start(qdup[H1, C0], qk_sb[H0, C0])
        nc.sync.dma_start(qdup[H1, C1], qk_sb[H0, C1])

        # ---- phase C: chunked attention + output projection, batches interleaved ----
        with (
            tc.tile_pool(name="vk", bufs=4) as vkp,
            tc.tile_pool(name="ssb", bufs=3) as ssbp,
            tc.tile_pool(name="osb", bufs=3) as osbp,
            tc.tile_pool(name="stsb", bufs=1) as stsbp,
            tc.tile_pool(name="ysb", bufs=3) as ysbp,
            tc.tile_pool(name="psA", bufs=3, space="PSUM") as psA,
            tc.tile_pool(name="psB", bufs=3, space="PSUM") as psB,
            tc.tile_pool(name="psY", bufs=2, space="PSUM") as psY,
        ):
            state_sb = [stsbp.tile([64, 64], F32, tag=f"state{b}",
                                   name=f"state_sb{b}") for b in range(B)]
            for t in range(NBLK):
                for b in range(B):
                    cols = bass.ts(b * NBLK + t, BLK)
                    # one transpose yields [v'tm | k'tm] (v_sb rows: 0:64 v', 64:128 k')
                    tr_ps = psA.tile([128, 128], F32, tag="blk")
                    nc.tensor.transpose(tr_ps[:], v_sb[:, cols], ident_sb[:])
                    vk = vkp.tile([128, 128], F32)
                    if (t + b) % 2 == 0:
                        nc.vector.tensor_copy(vk[:], tr_ps[:])
                    else:
                        nc.scalar.copy(vk[:], tr_ps[:])
                    # S^T = k' q'^T on this block, masked to kt<=qt
                    s_ps = psA.tile([128, BLK], F32, tag="blk")
                    nc.tensor.matmul(s_ps[:], v_sb[64:128, cols], qdup[64:128, cols],
                                     start=True, stop=True)
                    ssb = ssbp.tile([128, BLK], F32)
                    nc.vector.tensor_mul(ssb[:], s_ps[:], mask_sb[:])
                    # O^T = State^T q'^T (inter) + V^T S^T (intra)
                    o_ps = psB.tile([64, BLK], F32, tag="ob")
                    if t > 0:
                        nc.tensor.matmul(o_ps[:], state_sb[b][:], qk_sb[0:64, cols],
                                         start=True, stop=False)
                    nc.tensor.matmul(o_ps[:], vk[:, 0:64], ssb[:],
                                     start=(t == 0), stop=True)
                    # State += K'^T V, accumulated in SBUF
                    if t < NBLK - 1:
                        st_ps = psB.tile([64, 64], F32, tag="ob")
                        nc.tensor.matmul(st_ps[:], vk[:, 64:128], vk[:, 0:64],
                                         start=True, stop=True)
                        if t == 0:
                            nc.vector.tensor_copy(state_sb[b][:], st_ps[:])
                        else:
                            nc.vector.tensor_add(state_sb[b][:], state_sb[b][:],
                                                 st_ps[:])
                    osb = osbp.tile([64, BLK], F32)
                    nc.scalar.copy(osb[:], o_ps[:])
                    # y = O @ wout_h   [128 tok, 512]
                    y_ps = psY.tile([128, DIM], F32)
                    nc.tensor.matmul(y_ps[:], osb[:], wout_sb[:], start=True, stop=True)
                    ysb = ysbp.tile([128, DIM], F32)
                    if (t + b) % 2 == 0:
                        nc.vector.tensor_copy(ysb[:], y_ps[:])
                    else:
                        nc.scalar.copy(ysb[:], y_ps[:])
                    r0 = b * N + t * BLK
                    nc.sync.dma_start(y_d[r0:r0 + BLK, :], ysb[:])
    nc.finalize()
    return nc


_NC_CACHE = {}


def _get_nc(with_qkv_bias: bool):
    if with_qkv_bias not in _NC_CACHE:
        _NC_CACHE[with_qkv_bias] = build_nc(with_qkv_bias)
    return _NC_CACHE[with_qkv_bias]


def make_in_maps(x, gamma, w_qkv, b_qkv, w_a, b_a, w_out, b_out, with_qkv_bias):
    x = np.asarray(x, np.float32)
    gamma = np.asarray(gamma, np.float32)
    w_qkv = np.asarray(w_qkv, np.float32)
    b_qkv = np.asarray(b_qkv, np.float32)
    w_a = np.asarray(w_a, np.float32)
    b_a = np.asarray(b_a, np.float32)

    xT = np.ascontiguousarray(x.reshape(NTOK, DIM).T)
    wq = w_qkv[:, 0:DIM] * gamma[:, None] * SCALE
    wk = w_qkv[:, DIM:2 * DIM] * gamma[:, None]
    wv = w_qkv[:, 2 * DIM:3 * DIM] * gamma[:, None]
    wa = w_a * gamma[:, None]
    ident = np.eye(128, dtype=np.float32)
    mask = np.triu(np.ones((128, 128), np.float32))  # [kt, qt] keep kt<=qt

    in_maps = []
    for h in range(HEADS):
        sl = slice(h * DH, (h + 1) * DH)
        wqk = np.concatenate([wq[:, sl], wk[:, sl]], axis=1)   # [512, 128]
        wvz = np.concatenate([wv[:, sl], wa[:, sl]], axis=1)   # [512, 128]
        m = {
            "xT": xT,
            "wqk": np.ascontiguousarray(wqk.reshape(NCHUNK, 128, 128).transpose(1, 0, 2)),
            "wvz": np.ascontiguousarray(wvz.reshape(NCHUNK, 128, 128).transpose(1, 0, 2)),
            "wout": np.ascontiguousarray(np.asarray(w_out, np.float32)[sl, :]),
            "ba": np.ascontiguousarray(np.tile(b_a[sl], 2)[:, None]),
            "nba": np.ascontiguousarray(np.tile(-b_a[sl], 2)[:, None]),
            "ident": ident,
            "mask": mask,
        }
        if with_qkv_bias:
            bq = b_qkv[0:DIM][sl] * SCALE
            bk = b_qkv[DIM:2 * DIM][sl]
            bv = b_qkv[2 * DIM:3 * DIM][sl]
            m["bqk"] = np.ascontiguousarray(
                np.concatenate([bq, bk])[:, None].astype(np.float32))
            m["bv"] = np.ascontiguousarray(bv[:, None].astype(np.float32))
        in_maps.append(m)
    return in_maps


def kernel(x, gamma, w_qkv, b_qkv, w_a, b_a, w_out, b_out, _profile=None):
    with_qkv_bias = bool(np.any(np.asarray(b_qkv)))
    nc = _get_nc(with_qkv_bias)
    in_maps = make_in_maps(x, gamma, w_qkv, b_qkv, w_a, b_a, w_out, b_out,
                           with_qkv_bias)
    kwargs = dict(_profile) if _profile else {}
    res = run_bass_kernel_spmd(nc, in_maps, core_ids=list(range(HEADS)), **kwargs)
    if _profile is not None:
        _profile["result"] = res
    out = res.results[0]["ypart"].astype(np.float32).copy()
    for i in range(1, HEADS):
        out += res.results[i]["ypart"]
    out += np.asarray(b_out, np.float32)[None, :]
    return out.reshape(B, N, DIM)



# revision 13
# speedup vs baseline: 2.4861x; 2.4861x over previous
"""Trainium2 Bass kernel for nn_CausalFullAttention_13735305413109 (v2).

Causal attention with a data-dependent cumprod decay gate and no softmax;
algebraically a chunked linear attention.  All matmuls run in bf16 (1
cycle/row on the PE vs 4 for fp32); accumulation stays fp32 in PSUM.
Input x ships as bf16, output partial ships as fp16 (correctness gate is
rel_err < 2e-2).

Sharding: core = (batch, head-pair): b = cid//4, heads (2hp, 2hp+1).  Each
core handles 2048 tokens x 2 heads so the output projection contracts over
K=128 (both heads' 64 dims stacked via PSUM partition-offset matmuls) and
the host sums 4 partial projections per batch (+ b_out).

Scaling: no rms-scale s on q/k.  v carries s^2 (k-side and v-side s of
every j term) and the qt-side s_i is applied at the y PSUM->SBUF cast as a
per-partition activation scale (s transposed to token-major via a DMA
transpose).  q'' = q*a_cum; k'' = k*min(1/a_cum, 1e8) - identical to
1/max(a_cum, 1e-8).  cumprod via per-head DVE scans segmented per panel,
chained with an AP carry, overlapped with the projection matmuls.
"""
import numpy as np
import ml_dtypes
from contextlib import ExitStack

import concourse.bass as bass
import concourse.bacc as bacc
import concourse.mybir as mybir
import concourse.tile as tile
from concourse.bass_utils import run_bass_kernel_spmd

F32 = mybir.dt.float32
BF16 = mybir.dt.bfloat16
FP16 = mybir.dt.float16
AF = mybir.ActivationFunctionType
ALU = mybir.AluOpType

B = 2
N = 2048                 # tokens per batch (= per core)
DIM = 512
HEADS = 8
DH = 64
NCHUNK = 4               # 512 features / 128
PANEL = 512
NPAN = N // PANEL        # 4
BLK = 128
NBLK = N // BLK          # 16
SCALE = DH ** -0.5
LOG_SQRT_DIM = float(np.log(np.sqrt(DIM)))
INV_EPS = 1e8


def build_nc(with_qkv_bias: bool):
    nc = bacc.Bacc()
    xT_d = nc.dram_tensor("xT", [128, NCHUNK, N], BF16, kind="ExternalInput")
    wqk_d = nc.dram_tensor("wqk", [128, 2, NCHUNK, 128], BF16, kind="ExternalInput")
    wvz_d = nc.dram_tensor("wvz", [128, 2, NCHUNK, 128], BF16, kind="ExternalInput")
    wout_d = nc.dram_tensor("wout", [128, DIM], BF16, kind="ExternalInput")
    bah_d = nc.dram_tensor("bah", [128, 2], F32, kind="ExternalInput")
    mask_d = nc.dram_tensor("mask", [128, 4, 128], BF16, kind="ExternalInput")
    if with_qkv_bias:
        bqk_d = nc.dram_tensor("bqk", [128, 2], F32, kind="ExternalInput")
        bv_d = nc.dram_tensor("bv", [64, 2], F32, kind="ExternalInput")
    y_d = nc.dram_tensor("ypart", [N, DIM], FP16, kind="ExternalOutput")

    with tile.TileContext(nc) as tc, ExitStack() as ctx:
        per = ctx.enter_context(tc.tile_pool(name="persist", bufs=1))
        wqk_sb = per.tile([128, 2, NCHUNK, 128], BF16, tag="wqk")
        wvz_sb = per.tile([128, 2, NCHUNK, 128], BF16, tag="wvz")
        wout_sb = per.tile([128, DIM], BF16, tag="wout")
        bah_sb = per.tile([128, 2], F32, tag="bah")
        mask_sb = per.tile([128, 4, 128], BF16, tag="mask")
        ones_sb = per.tile([128, 128], BF16, tag="ones")
        lsd_sb = per.tile([128, 1], F32, tag="lsd")
        lsd2_sb = per.tile([128, 1], F32, tag="lsd2")
        # persistent activations
        vz2 = per.tile([128, NPAN, 2, PANEL], BF16, tag="vz2")   # v''| z*s
        qk2 = per.tile([128, NPAN, 2, PANEL], BF16, tag="qk2")   # q''| k''
        vkT = per.tile([128, NPAN, 2, NCHUNK, 128], BF16, tag="vkT")
        qup = per.tile([128, 2, N], BF16, tag="qup")             # q'' rows 64:128
        acum = per.tile([128, 2, N], BF16, tag="acum")           # rows 64:128
        s21 = per.tile([128, N], F32, tag="s21")        # rows 0:64 s^2, 64:128 s
        s16 = per.tile([80, N], BF16, tag="s16")        # rows 64:80 = s (bf16)
        sT = per.tile([128, NBLK, 16], BF16, tag="sT")           # s token-major
        sTf = per.tile([128, NBLK, 16], F32, tag="sTf")
        state_sb = per.tile([128, DH], F32, tag="state_sb")
        state_bf = per.tile([128, NBLK - 1, DH], BF16, tag="state_bf")

        nc.sync.dma_start(wqk_sb[:], wqk_d[:])
        nc.sync.dma_start(wvz_sb[:], wvz_d[:])
        nc.sync.dma_start(wout_sb[:], wout_d[:])
        nc.sync.dma_start(bah_sb[:], bah_d[:])
        nc.sync.dma_start(mask_sb[:], mask_d[:])
        if with_qkv_bias:
            bqk_sb = per.tile([128, 2], F32, tag="bqk")
            bv_sb = per.tile([64, 2], F32, tag="bv")
            nc.sync.dma_start(bqk_sb[:], bqk_d[:])
            nc.sync.dma_start(bv_sb[:], bv_d[:])
        nc.gpsimd.memset(ones_sb[:], 1.0)
        nc.gpsimd.memset(lsd_sb[:], LOG_SQRT_DIM)
        nc.gpsimd.memset(lsd2_sb[:], 2.0 * LOG_SQRT_DIM)
        if with_qkv_bias:
            nlsd_sb = per.tile([128, 1], F32, tag="nlsd")
            nc.gpsimd.memset(nlsd_sb[:], -LOG_SQRT_DIM)

        # ---- phase A/B: projections + decay, pipelined per 512-token panel ----
        with (
            tc.tile_pool(name="xt", bufs=NPAN) as xtp,
            tc.tile_pool(name="x2", bufs=2) as x2p,
            tc.tile_pool(name="lns", bufs=2) as lnp,
            tc.tile_pool(name="at", bufs=2) as atp,
            tc.tile_pool(name="mac", bufs=2) as macp,
            tc.tile_pool(name="ss_ps", bufs=1, space="PSUM") as ssp,
            tc.tile_pool(name="vz_ps", bufs=1, space="PSUM") as vzp,
            tc.tile_pool(name="qk_ps", bufs=2, space="PSUM") as qkp,
        ):
            xts = []
            for p in range(NPAN):
                xt = xtp.tile([128, NCHUNK, PANEL], BF16, tag=f"xt{p}",
                              name=f"xt{p}")
                nc.sync.dma_start(xt[:], xT_d[:, :, bass.ts(p, PANEL)])
                xts.append(xt)
            for p in range(NPAN):
                cols = bass.ts(p, PANEL)
                xt = xts[p]
                x2 = x2p.tile([128, NCHUNK, PANEL], BF16)
                if p % 2 == 0:
                    nc.vector.tensor_mul(x2[:], xt[:], xt[:])
                else:
                    nc.gpsimd.tensor_mul(x2[:], xt[:], xt[:])
                ss_ps = ssp.tile([128, PANEL], F32)
                for c in range(NCHUNK):
                    nc.tensor.matmul(ss_ps[:], ones_sb[:], x2[:, c, :],
                                     start=(c == 0), stop=(c == NCHUNK - 1))
                lnt = lnp.tile([128, PANEL], F32)
                nc.scalar.activation(lnt[:], ss_ps[:], AF.Ln)
                # s^2 on rows 0:64 (v scale), s on rows 64:128 (z scale)
                nc.scalar.activation(s21[0:64, cols], lnt[0:64, :], AF.Exp,
                                     bias=lsd2_sb[0:64, :], scale=-1.0)
                nc.scalar.activation(s21[64:128, cols], lnt[64:128, :], AF.Exp,
                                     bias=lsd_sb[64:128, :], scale=-0.5)
                # s in bf16 token rows for the y-scale transpose
                nc.scalar.copy(s16[64:80, cols], s21[64:80, cols])

                vz_ps = vzp.tile([128, 2, PANEL], F32)
                for h in range(2):
                    for c in range(NCHUNK):
                        nc.tensor.matmul(vz_ps[:, h, :], wvz_sb[:, h, c, :],
                                         xt[:, c, :], start=(c == 0),
                                         stop=(c == NCHUNK - 1))
                s21b = s21[:, cols].unsqueeze(1).broadcast_to([128, 2, PANEL])
                # rows 0:64: v'' = v*s^2 ; rows 64:128: z*s   (one op)
                nc.vector.tensor_tensor(vz2[:, p, :, :], vz_ps[:], s21b,
                                        ALU.mult)
                if with_qkv_bias:
                    for h in range(2):
                        nc.vector.scalar_tensor_tensor(
                            vz2[0:64, p, h, :], s21[64:128, cols],
                            bv_sb[:, h:h + 1], vz2[0:64, p, h, :],
                            ALU.mult, ALU.add)

                qk_ps = qkp.tile([128, 2, PANEL], F32)
                for h in range(2):
                    for c in range(NCHUNK):
                        nc.tensor.matmul(qk_ps[:, h, :], wqk_sb[:, h, c, :],
                                         xt[:, c, :], start=(c == 0),
                                         stop=(c == NCHUNK - 1))
                if with_qkv_bias:
                    # repr lacks s: q_repr = q + bq/s, k_repr = k + bk/s
                    rs = lnp.tile([128, PANEL], F32, tag="rs")
                    nc.scalar.activation(rs[:], lnt[:], AF.Exp,
                                         bias=nlsd_sb[:], scale=0.5)
                    for h in range(2):
                        nc.vector.scalar_tensor_tensor(
                            qk_ps[:, h, :], rs[:], bqk_sb[:, h:h + 1],
                            qk_ps[:, h, :], ALU.mult, ALU.add)

                # ---- decay gate for this panel ----
                at = atp.tile([128, 2, PANEL], BF16)
                for h in range(2):
                    nc.scalar.activation(at[64:128, h, :], vz2[64:128, p, h, :],
                                         AF.Sigmoid, bias=bah_sb[64:128, h:h + 1])
                    init = (1.0 if p == 0
                            else acum[64:128, h, p * PANEL - 1:p * PANEL])
                    nc.vector.tensor_tensor_scan(
                        acum[64:128, h, cols], at[64:128, h, :],
                        at[64:128, h, :], init, ALU.mult, ALU.bypass)
                # combined multiplier: rows 0:64 = acum (for q), 64:128 = 1/acum
                mac = macp.tile([128, 2, PANEL], BF16)
                with nc.allow_low_precision("1/a_cum feeds bf16 matmul inputs"):
                    nc.vector.reciprocal(mac[64:128, :, :],
                                         acum[64:128, :, cols])
                nc.sync.dma_start(mac[0:64, :, :], acum[64:128, :, cols])
                # q'' = q*acum ; k'' = k*min(1/acum, 1e8)   (one fused op)
                nc.vector.scalar_tensor_tensor(qk2[:, p, :, :], mac[:],
                                               INV_EPS, qk_ps[:],
                                               ALU.min, ALU.mult)
                nc.sync.dma_start(qup[64:128, :, cols], qk2[0:64, p, :, :])
                nc.sync.dma_start_transpose(vkT[:, p, :, :, 0:64],
                                            vz2[0:64, p, :, :])
                nc.sync.dma_start_transpose(vkT[:, p, :, :, 64:128],
                                            qk2[64:128, p, :, :])
            nc.sync.dma_start_transpose(sT[:], s16[64:80, :])
            nc.scalar.copy(sTf[:], sT[:])

        # ---- phase C: state prefix + attention rounds (2 rounds merged) ----
        with (
            tc.tile_pool(name="ssb", bufs=2) as ssbp,
            tc.tile_pool(name="osb", bufs=2) as osbp,
            tc.tile_pool(name="ysb", bufs=2) as ysp,
            tc.tile_pool(name="st_ps", bufs=1, space="PSUM") as stp,
            tc.tile_pool(name="s_ps", bufs=2, space="PSUM") as sp_,
            tc.tile_pool(name="o_ps", bufs=2, space="PSUM") as op_,
            tc.tile_pool(name="y_ps", bufs=2, space="PSUM") as yp,
        ):
            # state prefix: state_bf[:, t, :] = sum_{tau<=t} K''^T V''
            for t in range(NBLK - 1):
                pseg, j = divmod(t, NPAN)
                st_ps = stp.tile([128, DH], F32)
                for h in range(2):
                    nc.tensor.matmul(st_ps[h * 64:(h + 1) * 64, :],
                                     vkT[:, pseg, h, j, 64:128],
                                     vkT[:, pseg, h, j, 0:64],
                                     start=True, stop=True)
                if t == 0:
                    nc.vector.tensor_copy(state_sb[:], st_ps[:])
                else:
                    nc.vector.tensor_add(state_sb[:], state_sb[:], st_ps[:])
                nc.scalar.copy(state_bf[:, t, :], state_sb[:])

            for tp in range(NBLK // 2):        # pair of rounds (2t, 2t+1)
                s_ps = sp_.tile([128, 2, 2, 128], F32)
                o_ps = op_.tile([128, 2, 128], F32)
                for par in range(2):
                    t = 2 * tp + par
                    pseg, j = divmod(t, NPAN)
                    colsN = bass.ts(t, BLK)
                    jc = slice(j * BLK, (j + 1) * BLK)
                    for h in range(2):
                        nc.tensor.matmul(s_ps[:, par, h, :],
                                         qk2[64:128, pseg, h, jc],
                                         qup[64:128, h, colsN],
                                         start=True, stop=True)
                ssb = ssbp.tile([128, 2, 2, 128], BF16)
                nc.vector.tensor_tensor(ssb[:], s_ps[:], mask_sb[:], ALU.mult)
                for par in range(2):
                    t = 2 * tp + par
                    pseg, j = divmod(t, NPAN)
                    colsN = bass.ts(t, BLK)
                    for h in range(2):
                        hrows = slice(h * 64, (h + 1) * 64)
                        nc.tensor.matmul(o_ps[hrows, par, :],
                                         vkT[:, pseg, h, j, 0:64],
                                         ssb[:, par, h, :],
                                         start=True, stop=(t == 0))
                        if t > 0:
                            rhs = (qk2[0:64, pseg, 0, j * BLK:(j + 1) * BLK]
                                   if h == 0 else qup[64:128, 1, colsN])
                            nc.tensor.matmul(o_ps[hrows, par, :],
                                             state_bf[hrows, t - 1, :], rhs,
                                             start=False, stop=True)
                osb = osbp.tile([128, 2, 128], BF16)
                nc.scalar.copy(osb[:], o_ps[:])
                ys = ysp.tile([128, 2, DIM], FP16)
                for par in range(2):
                    t = 2 * tp + par
                    y_ps = yp.tile([128, DIM], F32)
                    nc.tensor.matmul(y_ps[:], osb[:, par, :], wout_sb[:],
                                     start=True, stop=True)
                    # qt-side rms scale s_i via per-partition activation scale
                    nc.scalar.activation(ys[:, par, :], y_ps[:], AF.Copy,
                                         scale=sTf[:, t, 0:1])
                out_ap = y_d[2 * tp * BLK:(2 * tp + 2) * BLK, :].rearrange(
                    "(a b) c -> b a c", a=2)
                nc.scalar.dma_start(out_ap, ys[:])
    nc.finalize()
    return nc


_NC_CACHE = {}


def _get_nc(with_qkv_bias: bool):
    if with_qkv_bias not in _NC_CACHE:
        _NC_CACHE[with_qkv_bias] = build_nc(with_qkv_bias)
    return _NC_CACHE[with_qkv_bias]


def make_in_maps(x, gamma, w_qkv, b_qkv, w_a, b_a, w_out, b_out, with_qkv_bias):
    bf = ml_dtypes.bfloat16
    x = np.asarray(x, np.float32)
    gamma = np.asarray(gamma, np.float32)
    w_qkv = np.asarray(w_qkv, np.float32)
    b_qkv = np.asarray(b_qkv, np.float32)
    w_a = np.asarray(w_a, np.float32)
    b_a = np.asarray(b_a, np.float32)
    w_out = np.asarray(w_out, np.float32)

    wq = w_qkv[:, 0:DIM] * gamma[:, None] * SCALE
    wk = w_qkv[:, DIM:2 * DIM] * gamma[:, None]
    wv = w_qkv[:, 2 * DIM:3 * DIM] * gamma[:, None]
    wa = w_a * gamma[:, None]
    mask = np.triu(np.ones((128, 128), np.float32))        # [kt, qt] keep kt<=qt
    mask4 = np.stack([mask] * 4, axis=1).astype(bf)        # [128, 4, 128]

    xTs = []
    for b in range(B):
        xT = x[b].T.astype(bf)                              # [512, 2048]
        xTs.append(np.ascontiguousarray(
            xT.reshape(NCHUNK, 128, N).transpose(1, 0, 2)))

    in_maps = []
    for cid in range(HEADS):
        b, hp = divmod(cid, 4)
        heads = (2 * hp, 2 * hp + 1)

        def stack_w(wl, wr):
            per_h = []
            for h in heads:
                sl = slice(h * DH, (h + 1) * DH)
                cat = np.concatenate([wl[:, sl], wr[:, sl]], axis=1)  # [512,128]
                per_h.append(cat.reshape(NCHUNK, 128, 128).transpose(1, 0, 2))
            return np.ascontiguousarray(
                np.stack(per_h, axis=1).astype(bf))        # [128, 2, 4, 128]

        wout2 = np.concatenate(
            [w_out[h * DH:(h + 1) * DH, :] for h in heads], axis=0)
        bah = np.stack([np.tile(b_a[h * DH:(h + 1) * DH], 2) for h in heads],
                       axis=1)                              # [128, 2]
        m = {
            "xT": xTs[b],
            "wqk": stack_w(wq, wk),
            "wvz": stack_w(wv, wa),
            "wout": np.ascontiguousarray(wout2.astype(bf)),
            "bah": np.ascontiguousarray(bah.astype(np.float32)),
            "mask": mask4,
        }
        if with_qkv_bias:
            bqk = np.stack(
                [np.concatenate([b_qkv[h * DH:(h + 1) * DH] * SCALE,
                                 b_qkv[DIM + h * DH:DIM + (h + 1) * DH]])
                 for h in heads], axis=1)                   # [128, 2]
            bv = np.stack(
                [b_qkv[2 * DIM + h * DH:2 * DIM + (h + 1) * DH]
                 for h in heads], axis=1)                   # [64, 2]
            m["bqk"] = np.ascontiguousarray(bqk.astype(np.float32))
            m["bv"] = np.ascontiguousarray(bv.astype(np.float32))
        in_maps.append(m)
    return in_maps


def kernel(x, gamma, w_qkv, b_qkv, w_a, b_a, w_out, b_out, _profile=None):
    with_qkv_bias = bool(np.any(np.asarray(b_qkv)))
    nc = _get_nc(with_qkv_bias)
    in_maps = make_in_maps(x, gamma, w_qkv, b_qkv, w_a, b_a, w_out, b_out,
                           with_qkv_bias)
    kwargs = dict(_profile) if _profile else {}
    res = run_bass_kernel_spmd(nc, in_maps, core_ids=list(range(HEADS)), **kwargs)
    if _profile is not None:
        _profile["result"] = res
    out = np.empty((B, N, DIM), np.float32)
    for b in range(B):
        acc = np.zeros((N, DIM), np.float32)
        for hp in range(4):
            acc += np.asarray(res.results[b * 4 + hp]["ypart"], np.float32)
        out[b] = acc
    out += np.asarray(b_out, np.float32)[None, None, :]
    return out
